# revision 1
# baseline (speedup 1.0000x reference)
"""Trainium2 Bass kernel for nn_DNCClassifier_82635170775168.

Key observation: in the reference DNC, the controller input is
``cat(x_t, zeros)`` every step (the ixaxaar dnc.py bug: read vectors are
never fed back), so the LSTM state (h, c) evolves independently of the
DNC memory subsystem, and the output ``h_T @ W_fc.T + b_fc`` depends only
on the LSTM path.  The external-memory machinery (usage, allocation,
temporal links, read weights) is dead code w.r.t. the output, so this
kernel computes just the LSTM recurrence + final linear layer.

Sharding: pure data parallel, batch 128 -> 16 per core across 8 cores.

Per-core design (feature-major: gate rows on partitions, batch on free;
gate rows permuted to chunk order [i, f, o, g]):
  - per step t, gate preactivations land in two psum banks (g separate
    from i/f/o).  Each bank is first seeded by an identity matmul with the
    precomputed x-projection U[t] (start=True; no h dependency, so it runs
    during the previous step's tail), then 16 accumulating bf16 weight
    matmuls  W_hh.T[k-tile, chunk] @ h[k-tile]  are layered on top.
  - tanh(g) runs on ACT straight off its psum bank while the i/f/o
    matmuls still stream, writing into A[:, 0:32] where A = [tanh_g | c];
    one wide DVE multiply  A * [sigma_i | sigma_f]  plus one add then
    yields the new cell state.  Sigmoid is split so sigma_o (only needed
    for h after tanh(c)) stays off the critical path.
  - U[t] = W_x.T @ [x_t; 1] (ones row carries the bias) is precomputed by
    matmuls contracting over K=28 in float32r (full fp32 precision at
    1 cycle/row), trickled into the step loop's idle slots as 16
    per-block SBUF tiles; phase-1 blocks reuse the tiles once their
    phase-0 readers are done.
"""

import sys

if "/opt/trn_rl_repo" not in sys.path:
    sys.path.insert(0, "/opt/trn_rl_repo")

import numpy as np

B_FULL = 128
N_CORES = 8
B = B_FULL // N_CORES   # 16 batch per core
T = 512
H = 256
G = 4 * H               # 1024 gate rows
IN = 27
INX = IN + 1            # + ones row for bias
OUT = 128
NCHUNK = 8              # gate-row chunks of 128
TB = 32                 # precompute time-block (32 steps x 16 batch = 512 cols)

W_DTYPE = "bfloat16"    # dtype of W_hh tiles and h (recurrent matmul)
U_DTYPE = "float32"     # dtype of U and the identity matmul
# float32r streams the moving operand at full rate for N>=256 while keeping
# fp32 precision on hardware (verified: same rel err as float32 here)
X_DTYPE = "float32r"    # dtype of the xT/W_x operands of the precompute matmuls


def _mybir_dt(name):
    import concourse.mybir as mybir

    return getattr(mybir.dt, name)


def build(t_steps=T, w_dtype=W_DTYPE, u_dtype=U_DTYPE, repeat=1,
          x_dtype=X_DTYPE):
    """Builds the per-core Bass program. Returns the Bacc instance.

    repeat > 1 re-runs the recurrence loop (timing-only builds: the extra
    passes reuse U and carry the h/c state on, so outputs are meaningless
    but per-pass timing is identical)."""
    import concourse.mybir as mybir
    from concourse import bacc
    from concourse.tile import TileContext

    assert t_steps % (2 * TB) == 0
    tph = t_steps // 2          # steps per phase
    nblk = tph // TB            # time blocks per phase

    fp32 = mybir.dt.float32
    wdt = _mybir_dt(w_dtype)
    udt = _mybir_dt(u_dtype)
    xdt = _mybir_dt(x_dtype)
    AFT = mybir.ActivationFunctionType
    ALU = mybir.AluOpType

    nc = bacc.Bacc("TRN2")

    d_xT = nc.dram_tensor("xT", [INX, t_steps * B], xdt, kind="ExternalInput")
    d_whh = nc.dram_tensor("whh", [128, 16 * 128], wdt, kind="ExternalInput")
    d_wx = nc.dram_tensor("wx", [INX, G], xdt, kind="ExternalInput")
    d_ident = nc.dram_tensor("ident", [128, 128], udt, kind="ExternalInput")
    d_wfc = nc.dram_tensor("wfc", [128, 2 * 128], fp32, kind="ExternalInput")
    d_bfc = nc.dram_tensor("bfc", [128, 1], fp32, kind="ExternalInput")
    d_y = nc.dram_tensor("y", [OUT, B], fp32, kind="ExternalOutput")

    with TileContext(nc) as tc:
        with (
            tc.tile_pool(name="persist", bufs=1) as persist,
            tc.tile_pool(name="state", bufs=2) as state,
            tc.tile_pool(name="work", bufs=3) as work,
            tc.tile_pool(name="pp_pre", bufs=2, space="PSUM") as pp_pre,
            tc.tile_pool(name="pp_g", bufs=2, space="PSUM") as pp_g,
            tc.tile_pool(name="pp_ifo", bufs=2, space="PSUM") as pp_ifo,
            tc.tile_pool(name="pp_fc", bufs=1, space="PSUM") as pp_fc,
        ):
            s_xT = persist.tile([INX, t_steps * B], xdt)
            s_whh = persist.tile([128, 16 * 128], wdt)
            s_wx = persist.tile([INX, G], xdt)
            s_ident = persist.tile([128, 128], udt)
            s_wfc = persist.tile([128, 2 * 128], fp32)
            s_bfc = persist.tile([128, 1], fp32)
            # one U tile per 32-step block so phase-1 blocks can be
            # recomputed into them as soon as phase-0 readers finish
            u_tiles = [
                persist.tile([128, TB * 128], udt, tag=f"U{tb}", name=f"U{tb}")
                for tb in range(nblk)
            ]

            nc.sync.dma_start(out=s_xT[:], in_=d_xT[:])
            nc.sync.dma_start(out=s_whh[:], in_=d_whh[:])
            nc.sync.dma_start(out=s_wx[:], in_=d_wx[:])
            nc.sync.dma_start(out=s_ident[:], in_=d_ident[:])
            nc.sync.dma_start(out=s_wfc[:], in_=d_wfc[:])
            nc.sync.dma_start(out=s_bfc[:], in_=d_bfc[:])

            h_cur = state.tile([128, 32], wdt, tag="h")
            # A holds [tanh(g) | c] so one wide DVE mul against the adjacent
            # [sigma_i | sigma_f] columns of sig yields both cell products
            A_cur = state.tile([128, 64], fp32, tag="A")
            nc.vector.memset(h_cur[:], 0.0)
            nc.vector.memset(A_cur[:], 0.0)

            def precompute_block(phase, tb):
                # U[t] for the 32 steps of block (phase, tb) into u_tiles[tb]
                t0 = phase * tph + tb * TB
                rhs = s_xT[:, t0 * B : (t0 + TB) * B]
                U4 = u_tiles[tb][:].rearrange(
                    "p (t c b) -> p t c b", c=NCHUNK, b=B
                )
                for c in range(NCHUNK):
                    ps = pp_pre.tile([128, TB * B], fp32, tag="ps_pre")
                    nc.tensor.matmul(
                        ps[:],
                        s_wx[:, c * 128 : (c + 1) * 128],
                        rhs,
                        start=True,
                        stop=True,
                    )
                    psv = ps[:].rearrange("p (t b) -> p t b", b=B)
                    # split the psum evacuation so an interleaved copy can
                    # only delay the step chain by ~half a copy
                    for half in range(2):
                        dst = U4[:, tb_half(half), c, :]
                        src = psv[:, tb_half(half), :]
                        if (c + half) % 2 == 0:
                            nc.vector.tensor_copy(out=dst, in_=src)
                        else:
                            nc.scalar.copy(out=dst, in_=src)

            def tb_half(half):
                return slice(half * (TB // 2), (half + 1) * (TB // 2))

            def step(tl):
                nonlocal h_cur, A_cur
                # g chunks get their own psum bank so tanh(g) runs on ACT
                # while the i,f,o matmuls are still streaming
                ps_g = pp_g.tile([128, 32], fp32, tag="ps_g")
                ps_ifo = pp_ifo.tile([128, 96], fp32, tag="ps_ifo")
                # identity matmuls lay down U[t] (+bias); no h dependency
                ublk = u_tiles[tl // TB]
                off = (tl % TB) * 128
                # both identity matmuls first: they have no h dependency, so
                # they run during the previous step's tail and never sit in
                # the PE FIFO between h's arrival and the weight matmuls
                nc.tensor.matmul(
                    ps_g[:], s_ident[:],
                    ublk[:, off + 96 : off + 128],
                    start=True, stop=False,
                )
                nc.tensor.matmul(
                    ps_ifo[:], s_ident[:],
                    ublk[:, off : off + 96],
                    start=True, stop=False,
                )
                for ci, c in enumerate((6, 7)):
                    for kt in range(2):
                        nc.tensor.matmul(
                            ps_g[:, ci * B : (ci + 1) * B],
                            s_whh[:, (kt * 8 + c) * 128 : (kt * 8 + c + 1) * 128],
                            h_cur[:, kt * B : (kt + 1) * B],
                            start=False,
                            stop=(ci == 1 and kt == 1),
                            skip_group_check=True,
                        )
                for c in range(6):
                    for kt in range(2):
                        nc.tensor.matmul(
                            ps_ifo[:, c * B : (c + 1) * B],
                            s_whh[:, (kt * 8 + c) * 128 : (kt * 8 + c + 1) * 128],
                            h_cur[:, kt * B : (kt + 1) * B],
                            start=False,
                            stop=(c == 5 and kt == 1),
                            skip_group_check=True,
                        )
                # tanh(g) straight off psum into A[:, 0:32] (overlaps ifo MMs)
                nc.scalar.activation(A_cur[:, 0:32], ps_g[:], AFT.Tanh)
                sig = work.tile([128, 96], fp32, tag="sig")
                # sigma split: i,f are on the critical path (cell update);
                # sigma_o is only needed for h after tanh(c), so it hides
                # behind the DVE chain instead of widening the critical op
                nc.scalar.activation(sig[:, 0:64], ps_ifo[:, 0:64], AFT.Sigmoid)
                nc.scalar.activation(sig[:, 64:96], ps_ifo[:, 64:96], AFT.Sigmoid)

                # prod = [tanh_g | c] * [sigma_i | sigma_f] in one wide op
                prod = work.tile([128, 64], fp32, tag="prod")
                nc.vector.tensor_mul(out=prod[:], in0=A_cur[:], in1=sig[:, 0:64])
                A_new = state.tile([128, 64], fp32, tag="A")
                nc.vector.tensor_add(
                    out=A_new[:, 32:64], in0=prod[:, 0:32], in1=prod[:, 32:64]
                )
                th = work.tile([128, 32], fp32, tag="th")
                nc.scalar.activation(th[:], A_new[:, 32:64], AFT.Tanh)
                h_new = state.tile([128, 32], wdt, tag="h")
                nc.vector.tensor_mul(out=h_new[:], in0=sig[:, 64:96], in1=th[:])
                h_cur, A_cur = h_new, A_new

            # block (0,0) first; the rest trickle into step-loop idle slots.
            # Phase-1 blocks reuse u_tiles[tb]: emitted only after every
            # phase-0 step that reads the tile, so their WAR dependency is
            # already satisfied and they never head-of-line-block the chain.
            precompute_block(0, 0)
            pending = [(0, tb) for tb in range(1, nblk)] + [
                (1, tb) for tb in range(nblk)
            ]
            for g in range(t_steps):
                phase, tl = divmod(g, tph)
                if g % 4 == 2 and pending:
                    for i, blk in enumerate(pending):
                        ph_b, tb_b = blk
                        if ph_b == 0 or g >= (tb_b + 1) * TB + 1:
                            precompute_block(ph_b, tb_b)
                            pending.pop(i)
                            break
                step(tl)
            assert not pending, pending
            for _rep in range(repeat - 1):
                for g in range(t_steps):
                    step(g % tph)

            # ---- classifier head: logits[o, b] = W_fc @ h + b_fc
            ps_fc = pp_fc.tile([128, B], fp32)
            h_fc = h_cur
            if w_dtype != "float32":
                h_fc = work.tile([128, 32], fp32, tag="h_fc32")
                nc.vector.tensor_copy(out=h_fc[:], in_=h_cur[:])
            for kt in range(2):
                nc.tensor.matmul(
                    ps_fc[:],
                    s_wfc[:, kt * 128 : (kt + 1) * 128],
                    h_fc[:, kt * B : (kt + 1) * B],
                    start=(kt == 0),
                    stop=(kt == 1),
                )
            out_sb = work.tile([128, B], fp32, tag="out_sb")
            nc.scalar.activation(
                out_sb[:], ps_fc[:], AFT.Identity, bias=s_bfc[:, 0:1]
            )
            nc.sync.dma_start(out=d_y[:], in_=out_sb[:])

    nc.compile()
    return nc


def prep_core_inputs(x, W_ih, W_hh, b_ih, b_hh, W_fc, b_fc, t_steps=T,
                     w_dtype=W_DTYPE, u_dtype=U_DTYPE, x_dtype=X_DTYPE):
    """Host-side layout prep. Returns list of per-core input dicts."""
    import ml_dtypes

    def npdt(name):
        return ml_dtypes.bfloat16 if name == "bfloat16" else np.float32

    x = np.ascontiguousarray(np.asarray(x, dtype=np.float32))
    W_ih = np.asarray(W_ih, dtype=np.float32)
    W_hh = np.asarray(W_hh, dtype=np.float32)
    bias = np.asarray(b_ih, dtype=np.float32) + np.asarray(b_hh, dtype=np.float32)
    W_fc = np.asarray(W_fc, dtype=np.float32)
    b_fc = np.asarray(b_fc, dtype=np.float32)

    # gate-row permutation: torch order [i, f, g, o] -> chunk order [i, f, o, g]
    perm = np.r_[0 : 2 * H, 3 * H : 4 * H, 2 * H : 3 * H]
    Wp_hh = W_hh[perm].copy()         # (1024, 256)
    Wp_ihx = W_ih[perm, :IN].copy()   # (1024, 27)
    bias_p = bias[perm].copy()        # (1024,)

    whh_host = np.empty((128, 16 * 128), dtype=np.float32)
    for kt in range(2):
        for c in range(NCHUNK):
            blk = Wp_hh[c * 128 : (c + 1) * 128, kt * 128 : (kt + 1) * 128].T
            whh_host[:, (kt * 8 + c) * 128 : (kt * 8 + c + 1) * 128] = blk
    whh_host = whh_host.astype(npdt(w_dtype))

    wx_host = np.empty((INX, G), dtype=np.float32)
    wx_host[:IN] = Wp_ihx.T
    wx_host[IN] = bias_p
    wx_host = wx_host.astype(npdt(x_dtype))

    ident_host = np.eye(128, dtype=np.float32).astype(npdt(u_dtype))

    wfc_host = np.empty((128, 2 * 128), dtype=np.float32)
    for kt in range(2):
        wfc_host[:, kt * 128 : (kt + 1) * 128] = W_fc[:, kt * 128 : (kt + 1) * 128].T
    bfc_host = b_fc.reshape(128, 1)

    in_maps = []
    for core in range(N_CORES):
        xc = x[core * B : (core + 1) * B, :t_steps, :]        # (16, t, 27)
        xT = np.empty((INX, t_steps * B), dtype=np.float32)
        xT[:IN] = xc.transpose(2, 1, 0).reshape(IN, t_steps * B)
        xT[IN] = 1.0
        in_maps.append(
            dict(
                xT=np.ascontiguousarray(xT.astype(npdt(x_dtype))),
                whh=whh_host,
                wx=wx_host,
                ident=ident_host,
                wfc=wfc_host,
                bfc=bfc_host,
            )
        )
    return in_maps


_NC_CACHE = {}


def _get_nc(t_steps=T, w_dtype=W_DTYPE, u_dtype=U_DTYPE, repeat=1):
    key = (t_steps, w_dtype, u_dtype, repeat)
    if key not in _NC_CACHE:
        _NC_CACHE[key] = build(t_steps, w_dtype, u_dtype, repeat)
    return _NC_CACHE[key]


def kernel(**inputs):
    from concourse.bass_utils import run_bass_kernel_spmd

    nc = _get_nc()
    in_maps = prep_core_inputs(
        inputs["x"],
        inputs["W_ih"],
        inputs["W_hh"],
        inputs["b_ih"],
        inputs["b_hh"],
        inputs["W_fc"],
        inputs["b_fc"],
    )
    res = run_bass_kernel_spmd(nc, in_maps, core_ids=list(range(N_CORES)))
    out = np.empty((B_FULL, OUT), dtype=np.float32)
    for core in range(N_CORES):
        out[core * B : (core + 1) * B, :] = res.results[core]["y"].T
    return out



# revision 2
# speedup vs baseline: 1.8044x; 1.8044x over previous
"""Trainium2 Bass kernel for nn_DNCClassifier_82635170775168 — v2.

Same dead-code insight as v1 (the DNC memory subsystem never feeds the
output; only the LSTM path matters).  v2 restructures the per-step
critical chain:

  - ALL four gates go through ONE sigmoid ACT instruction per step per
    group: tanh(g) = 2*sigmoid(2g) - 1, with the g-rows of W/U
    pre-scaled x2 so the psum bank holds [i | f | o | 2g] and a single
    (64+222)/1.2ns sigmoid covers everything (removes the serial
    tanh_g+sigmoid ACT pair of v1).
  - The cell state is kept as c2 = 2c and the hidden state as
    h'' = h/16, letting every elementwise op be one fused DVE
    scalar_tensor_tensor:
        p2  = sigma_f * c2
        q   = (sigma_2g - 0.5) * sigma_i        (= tanh(g)*sigma_i/2)
        c2' = 4q + p2                           (= 2 c_new)
        TH  = tanh(0.5 * c2')                   (ACT, scale port)
        h'' = (TH * 1/16) * sigma_o             (= h_new/16)
    The 1/16 compensates W' = 16*W_hh (better fp8 dynamic range); the
    classifier head uses W_fc' = 16*W_fc.
  - W_hh is stored as fp8e4m3 (CPU-checked: rel err 4e-3 vs the 2e-2
    gate): halves the real LDWEIGHTS cost per step (FWL loads fp8 4
    elems/cycle vs bf16 2).
  - The per-core batch of 16 is split into TWO groups of 8 whose
    dependence chains interleave: while group A waits on its
    ACT->DVE->ACT round trip, group B's instructions fill the engines.
  - U[t] (x-projection + bias) seeds each psum bank via ONE bf16
    identity matmul (N=64) instead of two fp32 ones.

Sharding: pure data parallel, batch 128 -> 16 per core across 8 cores.
"""

import sys

if "/opt/trn_rl_repo" not in sys.path:
    sys.path.insert(0, "/opt/trn_rl_repo")

import numpy as np

B_FULL = 128
N_CORES = 8
B = B_FULL // N_CORES   # 16 batch per core
G = 2                   # interleaved groups per core
BG = B // G             # 8 batch per group
T = 512
H = 256
IN = 27
INX = IN + 1            # + ones row for bias
OUT = 128
NCHUNK = 8              # gate-row chunks of 128 (i0 i1 f0 f1 o0 o1 g0 g1)
TB = 32                 # precompute time-block
WSCALE = 16.0           # W_hh pre-scale; h state stored as h/WSCALE

W_DTYPE = "float8e4"    # stationary W_hh tiles
H_DTYPE = "bfloat16"    # h state (matmul moving operand)
U_DTYPE = "bfloat16"    # U tiles + identity (seed matmul)
X_DTYPE = "float32r"    # x-projection precompute operands


def _mybir_dt(name):
    import concourse.mybir as mybir

    return getattr(mybir.dt, name)


def build(t_steps=T, w_dtype=W_DTYPE, u_dtype=U_DTYPE, repeat=1,
          x_dtype=X_DTYPE, groups=G, evac_acts=2):
    import concourse.mybir as mybir
    from concourse import bacc
    from concourse.tile import TileContext

    assert t_steps % (2 * TB) == 0
    bg = B // groups            # batch per group
    tph = t_steps // 2          # steps per phase
    nblk = tph // TB            # time blocks per phase

    fp32 = mybir.dt.float32
    wdt = _mybir_dt(w_dtype)
    hdt = _mybir_dt(H_DTYPE)
    udt = _mybir_dt(u_dtype)
    xdt = _mybir_dt(x_dtype)
    AFT = mybir.ActivationFunctionType
    ALU = mybir.AluOpType

    nc = bacc.Bacc("TRN2")

    d_xT = nc.dram_tensor("xT", [INX, t_steps * B], xdt, kind="ExternalInput")
    d_whh = nc.dram_tensor("whh", [128, 16 * 128], wdt, kind="ExternalInput")
    d_wx = nc.dram_tensor("wx", [INX, 8 * 128], xdt, kind="ExternalInput")
    d_ident = nc.dram_tensor("ident", [128, 128], udt, kind="ExternalInput")
    d_wfc = nc.dram_tensor("wfc", [128, 2 * 128], hdt, kind="ExternalInput")
    d_bfc = nc.dram_tensor("bfc", [128, 1], fp32, kind="ExternalInput")
    d_y = nc.dram_tensor("y", [OUT, B], fp32, kind="ExternalOutput")

    with TileContext(nc) as tc:
        import contextlib

        gp_bufs = 2 if groups <= 2 else 1
        with contextlib.ExitStack() as stack:
            persist = stack.enter_context(tc.tile_pool(name="persist", bufs=1))
            state = stack.enter_context(tc.tile_pool(name="state", bufs=2))
            work = stack.enter_context(tc.tile_pool(name="work", bufs=3))
            pp_pre = stack.enter_context(
                tc.tile_pool(name="pp_pre", bufs=2, space="PSUM"))
            pp_g = [
                stack.enter_context(
                    tc.tile_pool(name=f"pp_g{g}", bufs=gp_bufs, space="PSUM"))
                for g in range(groups)
            ]
            pp_fc = stack.enter_context(
                tc.tile_pool(name="pp_fc", bufs=1, space="PSUM"))
            s_xT = persist.tile([INX, t_steps * B], xdt)
            s_whh = persist.tile([128, 16 * 128], wdt)
            s_wx = persist.tile([INX, 8 * 128], xdt)
            s_ident = persist.tile([128, 128], udt)
            s_wfc = persist.tile([128, 2 * 128], hdt)
            s_bfc = persist.tile([128, 1], fp32)
            # one U tile per 32-step block, layout [128, (c, t, b)] bf16
            u_tiles = [
                persist.tile([128, NCHUNK * TB * B], udt, tag=f"U{tb}",
                             name=f"U{tb}")
                for tb in range(nblk)
            ]

            nc.sync.dma_start(out=s_xT[:], in_=d_xT[:])
            nc.sync.dma_start(out=s_whh[:], in_=d_whh[:])
            nc.sync.dma_start(out=s_wx[:], in_=d_wx[:])
            nc.sync.dma_start(out=s_ident[:], in_=d_ident[:])
            nc.sync.dma_start(out=s_wfc[:], in_=d_wfc[:])
            nc.sync.dma_start(out=s_bfc[:], in_=d_bfc[:])

            h_cur = [None] * groups
            c2_cur = [None] * groups
            for g in range(groups):
                h_cur[g] = state.tile([128, 2 * bg], hdt, tag=f"h{g}", name=f"h{g}")
                c2_cur[g] = state.tile([128, 2 * bg], fp32, tag=f"c{g}", name=f"c{g}")
                nc.vector.memset(h_cur[g][:], 0.0)
                nc.vector.memset(c2_cur[g][:], 0.0)

            def precompute_mm(phase, tb, c):
                # U rows of chunk c for the 32 steps of block (phase, tb)
                t0 = phase * tph + tb * TB
                rhs = s_xT[:, t0 * B : (t0 + TB) * B]
                ps = pp_pre.tile([128, TB * B], fp32, tag="ps_pre", name="ps_pre")
                nc.tensor.matmul(
                    ps[:],
                    s_wx[:, c * 128 : (c + 1) * 128],
                    rhs,
                    start=True,
                    stop=True,
                )
                return ps

            def precompute_copy(ps, tb, c):
                # psum -> bf16 U evacuation.  Emitted ~2 steps after the
                # matmul so the semaphore wait is already satisfied when it
                # reaches the ACT/DVE sequencers (an unsatisfied wait parks
                # the whole in-order sequencer, stalling the step chain).
                # Quarter-copies bound the stall any single copy imposes.
                dst = u_tiles[tb][:, c * TB * B : (c + 1) * TB * B]
                q4 = TB * B // 4
                for k in range(4):
                    d = dst[:, k * q4 : (k + 1) * q4]
                    s = ps[:, k * q4 : (k + 1) * q4]
                    if k < 4 - evac_acts:
                        nc.vector.tensor_copy(out=d, in_=s)
                    else:
                        nc.scalar.copy(out=d, in_=s)

            def mm_pass(g, tl):
                """Seed + 16 weight matmuls for group g, step-in-phase tl."""
                P = pp_g[g].tile([128, NCHUNK * bg], fp32, tag=f"ps{g}", name=f"ps{g}")
                ublk = u_tiles[tl // TB]
                u4 = ublk[:].rearrange("p (c t b) -> p c t b", c=NCHUNK, b=B)
                # seed: psum[c, b] = U[c, t, group-cols]  (identity matmul)
                nc.tensor.matmul(
                    P[:],
                    s_ident[:],
                    u4[:, :, tl % TB, g * bg : (g + 1) * bg],
                    start=True,
                    stop=False,
                )
                h = h_cur[g]
                for c in range(NCHUNK):
                    for kt in range(2):
                        nc.tensor.matmul(
                            P[:, c * bg : (c + 1) * bg],
                            s_whh[:, (c * 2 + kt) * 128 : (c * 2 + kt + 1) * 128],
                            h[:, kt * bg : (kt + 1) * bg],
                            start=False,
                            stop=(c == NCHUNK - 1 and kt == 1),
                            skip_group_check=True,
                        )
                return P

            def act1(g, P):
                # one sigmoid over [i | f | o | 2g] -> S
                S = work.tile([128, NCHUNK * bg], fp32, tag=f"S{g}", name=f"S{g}")
                nc.scalar.activation(S[:], P[:], AFT.Sigmoid)
                return S

            def tail(g, S):
                nonlocal h_cur, c2_cur
                w = 2 * bg
                si = S[:, 0:w]
                sf = S[:, w : 2 * w]
                so = S[:, 2 * w : 3 * w]
                s2g = S[:, 3 * w : 4 * w]
                p2 = work.tile([128, w], fp32, tag=f"p2{g}", name=f"p2{g}")
                nc.vector.tensor_mul(out=p2[:], in0=sf, in1=c2_cur[g][:])
                q = work.tile([128, w], fp32, tag=f"q{g}", name=f"q{g}")
                nc.vector.scalar_tensor_tensor(
                    out=q[:], in0=s2g, scalar=0.5, in1=si,
                    op0=ALU.subtract, op1=ALU.mult,
                )
                c2n = state.tile([128, w], fp32, tag=f"c{g}", name=f"c{g}")
                nc.vector.scalar_tensor_tensor(
                    out=c2n[:], in0=q[:], scalar=4.0, in1=p2[:],
                    op0=ALU.mult, op1=ALU.add,
                )
                th = work.tile([128, w], fp32, tag=f"th{g}", name=f"th{g}")
                nc.scalar.activation(th[:], c2n[:], AFT.Tanh, scale=0.5)
                h_new = state.tile([128, w], hdt, tag=f"h{g}", name=f"h{g}")
                nc.vector.scalar_tensor_tensor(
                    out=h_new[:], in0=th[:], scalar=1.0 / WSCALE, in1=so,
                    op0=ALU.mult, op1=ALU.mult,
                )
                c2_cur[g] = c2n
                h_cur[g] = h_new

            # precompute block (0,0) fully up front; the rest trickle in
            for c in range(NCHUNK):
                precompute_copy(precompute_mm(0, 0, c), 0, c)
            pending = [
                (ph, tb, c)
                for ph in range(2)
                for tb in range(nblk)
                for c in range(NCHUNK)
                if not (ph == 0 and tb == 0)
            ]
            inflight = []  # (ps, tb, c) awaiting evacuation

            # Half-step skew: group B's tail for step t-1 is emitted between
            # group A's front half (matmuls+sigmoid) and A's tail for step t,
            # and B's front half comes after A's tail.  This pins the two
            # dependence chains ~half a period apart so each group's ACT/DVE
            # round trips overlap the other group's matmul/idle windows
            # instead of queueing behind them on the in-order engines.
            # Ring skew: after group g's front half (matmuls + sigmoid),
            # emit the tail of the previous group in the ring.  This pins
            # the G dependence chains ~1/G of a period apart so each
            # group's ACT->DVE->ACT round trips overlap the other groups'
            # matmul/idle windows on the in-order engines.
            S_pend = [None] * groups

            def step_all(tl, mid=None):
                if groups <= 2:
                    # A-front, B-tail(t-1), [precompute], A-tail(t), B-front
                    # — measured fastest emission order for two groups
                    P = mm_pass(0, tl)
                    S_A = act1(0, P)
                    if groups == 2 and S_pend[1] is not None:
                        tail(1, S_pend[1])
                    if mid is not None:
                        mid()
                    tail(0, S_A)
                    if groups == 2:
                        P = mm_pass(1, tl)
                        S_pend[1] = act1(1, P)
                    return
                if mid is not None:
                    mid()
                for g in range(groups):
                    P = mm_pass(g, tl)
                    S_new = act1(g, P)
                    prev = (g - 1) % groups
                    if S_pend[prev] is not None:
                        tail(prev, S_pend[prev])
                        S_pend[prev] = None
                    S_pend[g] = S_new

            def pre_slot(gstep):
                if gstep % 4 == 0 and pending:
                    for i, (ph_b, tb_b, c_b) in enumerate(pending):
                        if ph_b == 0 or gstep >= (tb_b + 1) * TB - 2:
                            ps = precompute_mm(ph_b, tb_b, c_b)
                            inflight.append((ps, tb_b, c_b))
                            pending.pop(i)
                            break
                if gstep % 4 == 2 and inflight:
                    precompute_copy(*inflight.pop(0))

            for gstep in range(t_steps):
                phase, tl = divmod(gstep, tph)
                step_all(tl, mid=lambda: pre_slot(gstep))
            assert not pending, pending
            while inflight:
                precompute_copy(*inflight.pop(0))

            for _rep in range(repeat - 1):
                for gstep in range(t_steps):
                    step_all(gstep % tph)
            for g in range(groups):
                if S_pend[g] is not None:
                    tail(g, S_pend[g])
                    S_pend[g] = None

            # classifier head: logits[o, b] = (16 W_fc) @ h'' + b_fc
            ps_fc = pp_fc.tile([128, B], fp32)
            for g in range(groups):
                for kt in range(2):
                    nc.tensor.matmul(
                        ps_fc[:, g * bg : (g + 1) * bg],
                        s_wfc[:, kt * 128 : (kt + 1) * 128],
                        h_cur[g][:, kt * bg : (kt + 1) * bg],
                        start=(kt == 0),
                        stop=(kt == 1),
                        skip_group_check=True,
                    )
            out_sb = work.tile([128, B], fp32, tag="out_sb")
            nc.scalar.activation(
                out_sb[:], ps_fc[:], AFT.Identity, bias=s_bfc[:, 0:1]
            )
            nc.sync.dma_start(out=d_y[:], in_=out_sb[:])

    nc.compile()
    return nc


def prep_core_inputs(x, W_ih, W_hh, b_ih, b_hh, W_fc, b_fc, t_steps=T,
                     w_dtype=W_DTYPE, u_dtype=U_DTYPE, x_dtype=X_DTYPE):
    """Host-side layout prep. Returns list of per-core input dicts."""
    import ml_dtypes

    NPDT = {
        "bfloat16": ml_dtypes.bfloat16,
        "float8e4": ml_dtypes.float8_e4m3fn,
        "float8e5": ml_dtypes.float8_e5m2,
        "float32": np.float32,
        "float32r": np.float32,
    }

    x = np.ascontiguousarray(np.asarray(x, dtype=np.float32))
    W_ih = np.asarray(W_ih, dtype=np.float32)
    W_hh = np.asarray(W_hh, dtype=np.float32)
    bias = np.asarray(b_ih, dtype=np.float32) + np.asarray(b_hh, dtype=np.float32)
    W_fc = np.asarray(W_fc, dtype=np.float32)
    b_fc = np.asarray(b_fc, dtype=np.float32)

    # gate-row permutation: torch order [i, f, g, o] rows ->
    # chunk order [i0 i1 f0 f1 o0 o1 g0 g1] (g last, x2 for the
    # tanh-via-sigmoid fold)
    perm = np.r_[0 : 2 * H, 3 * H : 4 * H, 2 * H : 3 * H]
    Wp_hh = W_hh[perm].copy()         # (1024, 256)
    Wp_ihx = W_ih[perm, :IN].copy()   # (1024, 27)
    bias_p = bias[perm].copy()        # (1024,)

    gscale = np.ones((8 * 128, 1), np.float32)
    gscale[6 * 128 :] = 2.0           # g-rows: psum holds 2g

    whh_host = np.empty((128, 16 * 128), dtype=np.float32)
    Ws = Wp_hh * (WSCALE * gscale)
    for c in range(NCHUNK):
        for kt in range(2):
            blk = Ws[c * 128 : (c + 1) * 128, kt * 128 : (kt + 1) * 128].T
            whh_host[:, (c * 2 + kt) * 128 : (c * 2 + kt + 1) * 128] = blk
    whh_host = whh_host.astype(NPDT[w_dtype])

    wx_host = np.empty((INX, 8 * 128), dtype=np.float32)
    wx_host[:IN] = (Wp_ihx * gscale).T
    wx_host[IN] = bias_p * gscale[:, 0]
    wx_host = wx_host.astype(NPDT[x_dtype])

    ident_host = np.eye(128, dtype=np.float32).astype(NPDT[u_dtype])

    wfc_host = np.empty((128, 2 * 128), dtype=np.float32)
    for kt in range(2):
        wfc_host[:, kt * 128 : (kt + 1) * 128] = (
            WSCALE * W_fc[:, kt * 128 : (kt + 1) * 128].T
        )
    wfc_host = wfc_host.astype(NPDT["bfloat16"])
    bfc_host = b_fc.reshape(128, 1)

    in_maps = []
    for core in range(N_CORES):
        xc = x[core * B : (core + 1) * B, :t_steps, :]        # (16, t, 27)
        xT = np.empty((INX, t_steps * B), dtype=np.float32)
        xT[:IN] = xc.transpose(2, 1, 0).reshape(IN, t_steps * B)
        xT[IN] = 1.0
        in_maps.append(
            dict(
                xT=np.ascontiguousarray(xT.astype(NPDT[x_dtype])),
                whh=whh_host,
                wx=wx_host,
                ident=ident_host,
                wfc=wfc_host,
                bfc=bfc_host,
            )
        )
    return in_maps


_NC_CACHE = {}


def _get_nc(t_steps=T, w_dtype=W_DTYPE, u_dtype=U_DTYPE, repeat=1,
            groups=G, evac_acts=2):
    key = (t_steps, w_dtype, u_dtype, repeat, groups, evac_acts)
    if key not in _NC_CACHE:
        _NC_CACHE[key] = build(t_steps, w_dtype, u_dtype, repeat,
                               groups=groups, evac_acts=evac_acts)
    return _NC_CACHE[key]


def kernel(**inputs):
    from concourse.bass_utils import run_bass_kernel_spmd

    nc = _get_nc()
    in_maps = prep_core_inputs(
        inputs["x"],
        inputs["W_ih"],
        inputs["W_hh"],
        inputs["b_ih"],
        inputs["b_hh"],
        inputs["W_fc"],
        inputs["b_fc"],
    )
    res = run_bass_kernel_spmd(nc, in_maps, core_ids=list(range(N_CORES)))
    out = np.empty((B_FULL, OUT), dtype=np.float32)
    for core in range(N_CORES):
        out[core * B : (core + 1) * B, :] = res.results[core]["y"].T
    return out


# revision 3
# speedup vs baseline: 2.2198x; 1.2302x over previous
"""Trainium2 Bass kernel for nn_DNCClassifier_82635170775168 — v2.

Same dead-code insight as v1 (the DNC memory subsystem never feeds the
output; only the LSTM path matters).  v2 restructures the per-step
critical chain:

  - ALL four gates go through ONE sigmoid ACT instruction per step per
    group: tanh(g) = 2*sigmoid(2g) - 1, with the g-rows of W/U
    pre-scaled x2 so the psum bank holds [i | f | o | 2g] and a single
    (64+222)/1.2ns sigmoid covers everything (removes the serial
    tanh_g+sigmoid ACT pair of v1).
  - The cell state is kept as c2 = 2c and the hidden state as
    h'' = h/16, letting every elementwise op be one fused DVE
    scalar_tensor_tensor:
        p2  = sigma_f * c2
        q   = (sigma_2g - 0.5) * sigma_i        (= tanh(g)*sigma_i/2)
        c2' = 4q + p2                           (= 2 c_new)
        TH  = tanh(0.5 * c2')                   (ACT, scale port)
        h'' = (TH * 1/16) * sigma_o             (= h_new/16)
    The 1/16 compensates W' = 16*W_hh (better fp8 dynamic range); the
    classifier head uses W_fc' = 16*W_fc.
  - W_hh is stored as fp8e4m3 (CPU-checked: rel err 4e-3 vs the 2e-2
    gate): halves the real LDWEIGHTS cost per step (FWL loads fp8 4
    elems/cycle vs bf16 2).
  - The per-core batch of 16 is split into TWO groups of 8 whose
    dependence chains interleave: while group A waits on its
    ACT->DVE->ACT round trip, group B's instructions fill the engines.
  - U[t] (x-projection + bias) seeds each psum bank via ONE bf16
    identity matmul (N=64) instead of two fp32 ones.

Sharding: pure data parallel, batch 128 -> 16 per core across 8 cores.

Measured (8 axon trn2 cores, repeat-diff method): rel err 4.04e-3;
recurrence ~550-610 ns/step on a quiet device (vs 2063-2229 for the
v1 baseline measured the same way), ~1300-2300 under contention.
G=2 beat G=1 (1722) and G=4 (962) on hardware; the interleave matters
far more than the TimelineSim cost model predicts (it models the step
chain at ~1900 ns regardless, overestimating sem/ACT-access latency).
"""

import sys

if "/opt/trn_rl_repo" not in sys.path:
    sys.path.insert(0, "/opt/trn_rl_repo")

import numpy as np

B_FULL = 128
N_CORES = 8
B = B_FULL // N_CORES   # 16 batch per core
G = 2                   # interleaved groups per core
BG = B // G             # 8 batch per group
T = 512
H = 256
IN = 27
INX = IN + 1            # + ones row for bias
OUT = 128
NCHUNK = 8              # gate-row chunks of 128 (i0 i1 f0 f1 o0 o1 g0 g1)
TB = 32                 # precompute time-block
WSCALE = 16.0           # W_hh pre-scale; h state stored as h/WSCALE

W_DTYPE = "float8e4"    # stationary W_hh tiles
H_DTYPE = "bfloat16"    # h state (matmul moving operand)
U_DTYPE = "bfloat16"    # U tiles + identity (seed matmul)
X_DTYPE = "float32r"    # x-projection precompute operands


def _mybir_dt(name):
    import concourse.mybir as mybir

    return getattr(mybir.dt, name)


def build(t_steps=T, w_dtype=W_DTYPE, u_dtype=U_DTYPE, repeat=1,
          x_dtype=X_DTYPE, groups=G, evac_acts=2):
    import concourse.mybir as mybir
    from concourse import bacc
    from concourse.tile import TileContext

    assert t_steps % (2 * TB) == 0
    bg = B // groups            # batch per group
    tph = t_steps // 2          # steps per phase
    nblk = tph // TB            # time blocks per phase

    fp32 = mybir.dt.float32
    wdt = _mybir_dt(w_dtype)
    hdt = _mybir_dt(H_DTYPE)
    udt = _mybir_dt(u_dtype)
    xdt = _mybir_dt(x_dtype)
    AFT = mybir.ActivationFunctionType
    ALU = mybir.AluOpType

    nc = bacc.Bacc("TRN2")

    d_xT = nc.dram_tensor("xT", [INX, t_steps * B], xdt, kind="ExternalInput")
    d_whh = nc.dram_tensor("whh", [128, 16 * 128], wdt, kind="ExternalInput")
    d_wx = nc.dram_tensor("wx", [INX, 8 * 128], xdt, kind="ExternalInput")
    d_ident = nc.dram_tensor("ident", [128, 128], udt, kind="ExternalInput")
    d_wfc = nc.dram_tensor("wfc", [128, 2 * 128], hdt, kind="ExternalInput")
    d_bfc = nc.dram_tensor("bfc", [128, 1], fp32, kind="ExternalInput")
    d_y = nc.dram_tensor("y", [OUT, B], fp32, kind="ExternalOutput")

    with TileContext(nc) as tc:
        import contextlib

        gp_bufs = 2 if groups <= 2 else 1
        with contextlib.ExitStack() as stack:
            persist = stack.enter_context(tc.tile_pool(name="persist", bufs=1))
            state = stack.enter_context(tc.tile_pool(name="state", bufs=2))
            work = stack.enter_context(tc.tile_pool(name="work", bufs=3))
            pp_pre = stack.enter_context(
                tc.tile_pool(name="pp_pre", bufs=2, space="PSUM"))
            pp_g = [
                stack.enter_context(
                    tc.tile_pool(name=f"pp_g{g}", bufs=gp_bufs, space="PSUM"))
                for g in range(groups)
            ]
            pp_fc = stack.enter_context(
                tc.tile_pool(name="pp_fc", bufs=1, space="PSUM"))
            s_xT = persist.tile([INX, t_steps * B], xdt)
            s_whh = persist.tile([128, 16 * 128], wdt)
            s_wx = persist.tile([INX, 8 * 128], xdt)
            s_ident = persist.tile([128, 128], udt)
            s_wfc = persist.tile([128, 2 * 128], hdt)
            s_bfc = persist.tile([128, 1], fp32)
            # one U tile per 32-step block, layout [128, (c, t, b)] bf16
            u_tiles = [
                persist.tile([128, NCHUNK * TB * B], udt, tag=f"U{tb}",
                             name=f"U{tb}")
                for tb in range(nblk)
            ]

            nc.sync.dma_start(out=s_xT[:], in_=d_xT[:])
            nc.sync.dma_start(out=s_whh[:], in_=d_whh[:])
            nc.sync.dma_start(out=s_wx[:], in_=d_wx[:])
            nc.sync.dma_start(out=s_ident[:], in_=d_ident[:])
            nc.sync.dma_start(out=s_wfc[:], in_=d_wfc[:])
            nc.sync.dma_start(out=s_bfc[:], in_=d_bfc[:])

            h_cur = [None] * groups
            c2_cur = [None] * groups
            for g in range(groups):
                h_cur[g] = state.tile([128, 2 * bg], hdt, tag=f"h{g}", name=f"h{g}")
                c2_cur[g] = state.tile([128, 2 * bg], fp32, tag=f"c{g}", name=f"c{g}")
                nc.vector.memset(h_cur[g][:], 0.0)
                nc.vector.memset(c2_cur[g][:], 0.0)

            def precompute_mm(phase, tb, c):
                # U rows of chunk c for the 32 steps of block (phase, tb)
                t0 = phase * tph + tb * TB
                rhs = s_xT[:, t0 * B : (t0 + TB) * B]
                ps = pp_pre.tile([128, TB * B], fp32, tag="ps_pre", name="ps_pre")
                nc.tensor.matmul(
                    ps[:],
                    s_wx[:, c * 128 : (c + 1) * 128],
                    rhs,
                    start=True,
                    stop=True,
                )
                return ps

            def precompute_copy(ps, tb, c):
                # psum -> bf16 U evacuation.  Emitted ~2 steps after the
                # matmul so the semaphore wait is already satisfied when it
                # reaches the ACT/DVE sequencers (an unsatisfied wait parks
                # the whole in-order sequencer, stalling the step chain).
                # Quarter-copies bound the stall any single copy imposes.
                dst = u_tiles[tb][:, c * TB * B : (c + 1) * TB * B]
                q4 = TB * B // 4
                for k in range(4):
                    d = dst[:, k * q4 : (k + 1) * q4]
                    s = ps[:, k * q4 : (k + 1) * q4]
                    if k < 4 - evac_acts:
                        nc.vector.tensor_copy(out=d, in_=s)
                    else:
                        nc.scalar.copy(out=d, in_=s)

            def mm_pass(g, tl):
                """Seed + 16 weight matmuls for group g, step-in-phase tl."""
                P = pp_g[g].tile([128, NCHUNK * bg], fp32, tag=f"ps{g}", name=f"ps{g}")
                ublk = u_tiles[tl // TB]
                u4 = ublk[:].rearrange("p (c t b) -> p c t b", c=NCHUNK, b=B)
                # seed: psum[c, b] = U[c, t, group-cols]  (identity matmul)
                nc.tensor.matmul(
                    P[:],
                    s_ident[:],
                    u4[:, :, tl % TB, g * bg : (g + 1) * bg],
                    start=True,
                    stop=False,
                )
                h = h_cur[g]
                for c in range(NCHUNK):
                    for kt in range(2):
                        nc.tensor.matmul(
                            P[:, c * bg : (c + 1) * bg],
                            s_whh[:, (c * 2 + kt) * 128 : (c * 2 + kt + 1) * 128],
                            h[:, kt * bg : (kt + 1) * bg],
                            start=False,
                            stop=(c == NCHUNK - 1 and kt == 1),
                            skip_group_check=True,
                        )
                return P

            def act1(g, P):
                # one sigmoid over [i | f | o | 2g] -> S
                S = work.tile([128, NCHUNK * bg], fp32, tag=f"S{g}", name=f"S{g}")
                nc.scalar.activation(S[:], P[:], AFT.Sigmoid)
                return S

            def tail(g, S):
                nonlocal h_cur, c2_cur
                w = 2 * bg
                si = S[:, 0:w]
                sf = S[:, w : 2 * w]
                so = S[:, 2 * w : 3 * w]
                s2g = S[:, 3 * w : 4 * w]
                p2 = work.tile([128, w], fp32, tag=f"p2{g}", name=f"p2{g}")
                nc.vector.tensor_mul(out=p2[:], in0=sf, in1=c2_cur[g][:])
                q = work.tile([128, w], fp32, tag=f"q{g}", name=f"q{g}")
                nc.vector.scalar_tensor_tensor(
                    out=q[:], in0=s2g, scalar=0.5, in1=si,
                    op0=ALU.subtract, op1=ALU.mult,
                )
                c2n = state.tile([128, w], fp32, tag=f"c{g}", name=f"c{g}")
                nc.vector.scalar_tensor_tensor(
                    out=c2n[:], in0=q[:], scalar=4.0, in1=p2[:],
                    op0=ALU.mult, op1=ALU.add,
                )
                th = work.tile([128, w], fp32, tag=f"th{g}", name=f"th{g}")
                nc.scalar.activation(th[:], c2n[:], AFT.Tanh, scale=0.5)
                h_new = state.tile([128, w], hdt, tag=f"h{g}", name=f"h{g}")
                nc.vector.scalar_tensor_tensor(
                    out=h_new[:], in0=th[:], scalar=1.0 / WSCALE, in1=so,
                    op0=ALU.mult, op1=ALU.mult,
                )
                c2_cur[g] = c2n
                h_cur[g] = h_new

            # precompute block (0,0) fully up front; the rest trickle in
            for c in range(NCHUNK):
                precompute_copy(precompute_mm(0, 0, c), 0, c)
            pending = [
                (ph, tb, c)
                for ph in range(2)
                for tb in range(nblk)
                for c in range(NCHUNK)
                if not (ph == 0 and tb == 0)
            ]
            inflight = []  # (ps, tb, c) awaiting evacuation

            # Half-step skew: group B's tail for step t-1 is emitted between
            # group A's front half (matmuls+sigmoid) and A's tail for step t,
            # and B's front half comes after A's tail.  This pins the two
            # dependence chains ~half a period apart so each group's ACT/DVE
            # round trips overlap the other group's matmul/idle windows
            # instead of queueing behind them on the in-order engines.
            # Ring skew: after group g's front half (matmuls + sigmoid),
            # emit the tail of the previous group in the ring.  This pins
            # the G dependence chains ~1/G of a period apart so each
            # group's ACT->DVE->ACT round trips overlap the other groups'
            # matmul/idle windows on the in-order engines.
            S_pend = [None] * groups

            def step_all(tl, mid=None):
                if groups <= 2:
                    # A-front, B-tail(t-1), [precompute], A-tail(t), B-front
                    # — measured fastest emission order for two groups
                    P = mm_pass(0, tl)
                    S_A = act1(0, P)
                    if groups == 2 and S_pend[1] is not None:
                        tail(1, S_pend[1])
                    if mid is not None:
                        mid()
                    tail(0, S_A)
                    if groups == 2:
                        P = mm_pass(1, tl)
                        S_pend[1] = act1(1, P)
                    return
                if mid is not None:
                    mid()
                for g in range(groups):
                    P = mm_pass(g, tl)
                    S_new = act1(g, P)
                    prev = (g - 1) % groups
                    if S_pend[prev] is not None:
                        tail(prev, S_pend[prev])
                        S_pend[prev] = None
                    S_pend[g] = S_new

            def pre_slot(gstep):
                if gstep % 4 == 0 and pending:
                    for i, (ph_b, tb_b, c_b) in enumerate(pending):
                        if ph_b == 0 or gstep >= (tb_b + 1) * TB - 2:
                            ps = precompute_mm(ph_b, tb_b, c_b)
                            inflight.append((ps, tb_b, c_b))
                            pending.pop(i)
                            break
                if gstep % 4 == 2 and inflight:
                    precompute_copy(*inflight.pop(0))

            for gstep in range(t_steps):
                phase, tl = divmod(gstep, tph)
                step_all(tl, mid=lambda: pre_slot(gstep))
            assert not pending, pending
            while inflight:
                precompute_copy(*inflight.pop(0))

            for _rep in range(repeat - 1):
                for gstep in range(t_steps):
                    step_all(gstep % tph)
            for g in range(groups):
                if S_pend[g] is not None:
                    tail(g, S_pend[g])
                    S_pend[g] = None

            # classifier head: logits[o, b] = (16 W_fc) @ h'' + b_fc
            ps_fc = pp_fc.tile([128, B], fp32)
            for g in range(groups):
                for kt in range(2):
                    nc.tensor.matmul(
                        ps_fc[:, g * bg : (g + 1) * bg],
                        s_wfc[:, kt * 128 : (kt + 1) * 128],
                        h_cur[g][:, kt * bg : (kt + 1) * bg],
                        start=(kt == 0),
                        stop=(kt == 1),
                        skip_group_check=True,
                    )
            out_sb = work.tile([128, B], fp32, tag="out_sb")
            nc.scalar.activation(
                out_sb[:], ps_fc[:], AFT.Identity, bias=s_bfc[:, 0:1]
            )
            nc.sync.dma_start(out=d_y[:], in_=out_sb[:])

    nc.compile()
    return nc


def prep_core_inputs(x, W_ih, W_hh, b_ih, b_hh, W_fc, b_fc, t_steps=T,
                     w_dtype=W_DTYPE, u_dtype=U_DTYPE, x_dtype=X_DTYPE):
    """Host-side layout prep. Returns list of per-core input dicts."""
    import ml_dtypes

    NPDT = {
        "bfloat16": ml_dtypes.bfloat16,
        "float8e4": ml_dtypes.float8_e4m3fn,
        "float8e5": ml_dtypes.float8_e5m2,
        "float32": np.float32,
        "float32r": np.float32,
    }

    x = np.ascontiguousarray(np.asarray(x, dtype=np.float32))
    W_ih = np.asarray(W_ih, dtype=np.float32)
    W_hh = np.asarray(W_hh, dtype=np.float32)
    bias = np.asarray(b_ih, dtype=np.float32) + np.asarray(b_hh, dtype=np.float32)
    W_fc = np.asarray(W_fc, dtype=np.float32)
    b_fc = np.asarray(b_fc, dtype=np.float32)

    # gate-row permutation: torch order [i, f, g, o] rows ->
    # chunk order [i0 i1 f0 f1 o0 o1 g0 g1] (g last, x2 for the
    # tanh-via-sigmoid fold)
    perm = np.r_[0 : 2 * H, 3 * H : 4 * H, 2 * H : 3 * H]
    Wp_hh = W_hh[perm].copy()         # (1024, 256)
    Wp_ihx = W_ih[perm, :IN].copy()   # (1024, 27)
    bias_p = bias[perm].copy()        # (1024,)

    gscale = np.ones((8 * 128, 1), np.float32)
    gscale[6 * 128 :] = 2.0           # g-rows: psum holds 2g

    whh_host = np.empty((128, 16 * 128), dtype=np.float32)
    Ws = Wp_hh * (WSCALE * gscale)
    for c in range(NCHUNK):
        for kt in range(2):
            blk = Ws[c * 128 : (c + 1) * 128, kt * 128 : (kt + 1) * 128].T
            whh_host[:, (c * 2 + kt) * 128 : (c * 2 + kt + 1) * 128] = blk
    whh_host = whh_host.astype(NPDT[w_dtype])

    wx_host = np.empty((INX, 8 * 128), dtype=np.float32)
    wx_host[:IN] = (Wp_ihx * gscale).T
    wx_host[IN] = bias_p * gscale[:, 0]
    wx_host = wx_host.astype(NPDT[x_dtype])

    ident_host = np.eye(128, dtype=np.float32).astype(NPDT[u_dtype])

    wfc_host = np.empty((128, 2 * 128), dtype=np.float32)
    for kt in range(2):
        wfc_host[:, kt * 128 : (kt + 1) * 128] = (
            WSCALE * W_fc[:, kt * 128 : (kt + 1) * 128].T
        )
    wfc_host = wfc_host.astype(NPDT["bfloat16"])
    bfc_host = b_fc.reshape(128, 1)

    in_maps = []
    for core in range(N_CORES):
        xc = x[core * B : (core + 1) * B, :t_steps, :]        # (16, t, 27)
        xT = np.empty((INX, t_steps * B), dtype=np.float32)
        xT[:IN] = xc.transpose(2, 1, 0).reshape(IN, t_steps * B)
        xT[IN] = 1.0
        in_maps.append(
            dict(
                xT=np.ascontiguousarray(xT.astype(NPDT[x_dtype])),
                whh=whh_host,
                wx=wx_host,
                ident=ident_host,
                wfc=wfc_host,
                bfc=bfc_host,
            )
        )
    return in_maps


_NC_CACHE = {}


def _get_nc(t_steps=T, w_dtype=W_DTYPE, u_dtype=U_DTYPE, repeat=1,
            groups=G, evac_acts=2):
    key = (t_steps, w_dtype, u_dtype, repeat, groups, evac_acts)
    if key not in _NC_CACHE:
        _NC_CACHE[key] = build(t_steps, w_dtype, u_dtype, repeat,
                               groups=groups, evac_acts=evac_acts)
    return _NC_CACHE[key]


def kernel(**inputs):
    from concourse.bass_utils import run_bass_kernel_spmd

    nc = _get_nc()
    in_maps = prep_core_inputs(
        inputs["x"],
        inputs["W_ih"],
        inputs["W_hh"],
        inputs["b_ih"],
        inputs["b_hh"],
        inputs["W_fc"],
        inputs["b_fc"],
    )
    res = run_bass_kernel_spmd(nc, in_maps, core_ids=list(range(N_CORES)))
    out = np.empty((B_FULL, OUT), dtype=np.float32)
    for core in range(N_CORES):
        out[core * B : (core + 1) * B, :] = res.results[core]["y"].T
    return out
